# revision 8
# baseline (speedup 1.0000x reference)
"""Trainium2 Bass kernel for batched progressive box adjustment (nms_detection).

Contract: kernel(boxes, gt_boxes) -> (all_boxes [B,6,N,4], all_sims [B,6,N,M]),
matching reference.reference(). Self-contained; shards batch B=64 across 8
NeuronCores (8 images per core).
"""

import numpy as np

B, N, M = 64, 1000, 100
ITERS = 5
NCORES = 8
BPC = B // NCORES      # images per core
P = 125                # SBUF partitions used (N = T * P)
T = 8                  # n-blocks per image
NIT = ITERS + 1        # sims slots

_CACHE = {}


def _build_module():
    import concourse.bacc as bacc
    import concourse.tile as tile
    import concourse.mybir as mybir
    from concourse.alu_op_type import AluOpType as op
    from concourse.masks import make_identity

    f32 = mybir.dt.float32
    X = mybir.AxisListType.X
    Relu = mybir.ActivationFunctionType.Relu

    nc = bacc.Bacc()
    bx_in = nc.dram_tensor("bx", [P, BPC * T, 4], f32, kind="ExternalInput")
    gbt_in = nc.dram_tensor("gbt", [P, BPC, 5, M], f32, kind="ExternalInput")
    gt5_in = nc.dram_tensor("gt5", [M, BPC, 5], f32, kind="ExternalInput")
    gt0_in = nc.dram_tensor("gt0", [P, BPC * T, 4], f32, kind="ExternalInput")
    sims = nc.dram_tensor("sims", [BPC, NIT, N, M], f32, kind="ExternalOutput")
    obox = nc.dram_tensor("obox", [BPC, ITERS, N, 4], f32, kind="ExternalOutput")

    HB = BPC // 2  # process iou in two image-halves to bound SBUF

    with tile.TileContext(nc) as tc:
        with (
            tc.tile_pool(name="const", bufs=1) as cpool,
            tc.tile_pool(name="boxes", bufs=2) as bpool,
            tc.tile_pool(name="work", bufs=1) as wpool,
            tc.tile_pool(name="iou", bufs=1) as ipool,
            tc.tile_pool(name="maskp", bufs=1) as mpool,
            tc.tile_pool(name="stage", bufs=2) as spool,
            tc.tile_pool(name="small", bufs=2) as tpool,
            tc.tile_pool(name="ps_mT", bufs=1, space="PSUM") as ps_mT,
            tc.tile_pool(name="ps_bgT", bufs=1, space="PSUM") as ps_bgT,
            tc.tile_pool(name="ps_bg", bufs=1, space="PSUM") as ps_bg,
        ):
            ident = cpool.tile([128, 128], f32)
            make_identity(nc, ident[:])
            G = cpool.tile([P, BPC, 5, M], f32)
            nc.sync.dma_start(G[:], gbt_in[:])
            GT5 = cpool.tile([M, BPC, 5], f32)
            nc.sync.dma_start(GT5[:], gt5_in[:])
            GT0 = cpool.tile([P, BPC * T, 4], f32)
            nc.sync.dma_start(GT0[:], gt0_in[:])

            cur = bpool.tile([P, BPC * T, 4], f32)
            nc.sync.dma_start(cur[:], bx_in[:])

            for it in range(NIT):
                # --- per-box areas, columnar [P, B*T] ---
                wa = tpool.tile([P, BPC * T], f32)
                nc.vector.tensor_tensor(wa[:], cur[:, :, 2], cur[:, :, 0], op=op.subtract)
                ha = tpool.tile([P, BPC * T], f32)
                nc.vector.tensor_tensor(ha[:], cur[:, :, 3], cur[:, :, 1], op=op.subtract)
                areaA = tpool.tile([P, BPC * T], f32)
                nc.vector.tensor_tensor(areaA[:], wa[:], ha[:], op=op.mult)

                iou = ipool.tile([P, BPC, T, M], f32)
                for h in range(2):
                    bs = slice(h * HB, (h + 1) * HB)
                    sh = [P, HB, T, M]

                    def gb(j):
                        return G[:, bs, j, :].unsqueeze(2).broadcast_to(sh)

                    def cb(c):
                        v = cur[:, :, c].rearrange("p (b t) -> p b t", b=BPC)
                        return v[:, bs, :].unsqueeze(3).broadcast_to(sh)

                    aab = areaA[:].rearrange("p (b t) -> p b t", b=BPC)[:, bs, :]
                    aab = aab.unsqueeze(3).broadcast_to(sh)

                    m1x = wpool.tile(sh, f32, tag="w0")
                    nc.vector.tensor_tensor(m1x[:], gb(2), cb(2), op=op.min)
                    m2x = wpool.tile(sh, f32, tag="w1")
                    nc.vector.tensor_tensor(m2x[:], gb(0), cb(0), op=op.max)
                    w0x = wpool.tile(sh, f32, tag="w2")
                    nc.vector.tensor_tensor(w0x[:], m1x[:], m2x[:], op=op.subtract)
                    wrx = wpool.tile(sh, f32, tag="w3")
                    nc.scalar.activation(wrx[:], w0x[:], Relu)
                    m1y = wpool.tile(sh, f32, tag="w0")
                    nc.vector.tensor_tensor(m1y[:], gb(3), cb(3), op=op.min)
                    m2y = wpool.tile(sh, f32, tag="w1")
                    nc.vector.tensor_tensor(m2y[:], gb(1), cb(1), op=op.max)
                    w0y = wpool.tile(sh, f32, tag="w2")
                    nc.vector.tensor_tensor(w0y[:], m1y[:], m2y[:], op=op.subtract)
                    inter = wpool.tile(sh, f32, tag="w0")
                    nc.vector.scalar_tensor_tensor(
                        inter[:], w0y[:], 0.0, wrx[:], op0=op.max, op1=op.mult
                    )
                    s = wpool.tile(sh, f32, tag="w1")
                    nc.vector.tensor_tensor(s[:], aab, gb(4), op=op.add)
                    union = wpool.tile(sh, f32, tag="w2")
                    nc.vector.scalar_tensor_tensor(
                        union[:], inter[:], -1.0, s[:], op0=op.mult, op1=op.add
                    )
                    recip = wpool.tile(sh, f32, tag="w3")
                    nc.vector.reciprocal(recip[:], union[:])
                    nc.vector.tensor_tensor(iou[:, bs, :, :], inter[:], recip[:], op=op.mult)

                # stream sims out (per image: DMA APs are limited to 3 free dims)
                for b in range(BPC):
                    nc.sync.dma_start(
                        sims[b, it].rearrange("(t p) m -> p t m", p=P), iou[:, b]
                    )

                if it == ITERS:
                    break

                # --- argmax over m + gather of best gt (PE matmul path) ---
                rmax = tpool.tile([P, BPC, T], f32)
                nc.vector.reduce_max(rmax[:], iou[:], axis=X)
                mask = mpool.tile([P, BPC, T, M], f32)
                nc.vector.tensor_tensor(
                    mask[:],
                    iou[:],
                    rmax[:].unsqueeze(3).broadcast_to([P, BPC, T, M]),
                    op=op.is_ge,
                )

                bg_ps = ps_bg.tile([P, BPC, T, 5], f32)
                for b in range(BPC):
                    mT_ps = ps_mT.tile([M, T, 128], f32)
                    for t in range(T):
                        nc.tensor.transpose(
                            mT_ps[:, t, :P], mask[:, b, t, :], ident[:P, :P]
                        )
                    mT = spool.tile([M, T, 128], f32, tag="mT")
                    nc.scalar.copy(mT[:], mT_ps[:])
                    bgT_ps = ps_bgT.tile([5, T, 128], f32)
                    for t in range(T):
                        nc.tensor.matmul(
                            bgT_ps[:, t, :P], GT5[:, b, :], mT[:, t, :P],
                            start=True, stop=True,
                        )
                    bgT = spool.tile([5, T, 128], f32, tag="bgT")
                    nc.scalar.copy(bgT[:], bgT_ps[:])
                    for t in range(T):
                        nc.tensor.transpose(
                            bg_ps[:, b, t, :], bgT[:, t, :P], ident[:5, :5]
                        )
                bg = tpool.tile([P, BPC, T, 5], f32)
                nc.scalar.copy(bg[:], bg_ps[:])

                # --- tie fix: rows with >1 max (all-zero rows) take gt[0] ---
                sel = tpool.tile([P, BPC, T], mybir.dt.int32)
                nc.vector.tensor_scalar(
                    sel[:], bg[:, :, :, 4], 1.5, None, op0=op.is_gt
                )
                g4 = tpool.tile([P, BPC * T, 4], f32)
                g4v = g4[:].rearrange("p (b t) c -> p b t c", b=BPC)
                nc.vector.tensor_copy(g4v, bg[:, :, :, 0:4])
                nc.vector.copy_predicated(
                    g4v,
                    sel[:].unsqueeze(3).broadcast_to([P, BPC, T, 4]),
                    GT0[:].rearrange("p (b t) c -> p b t c", b=BPC),
                )

                # --- box update (reference op order), pairs (x,y) batched ---
                c_lt = cur[:, :, 0:2]
                c_rb = cur[:, :, 2:4]
                g_lt = g4[:, :, 0:2]
                g_rb = g4[:, :, 2:4]
                sh2 = [P, BPC * T, 2]

                def w2(tag):
                    return tpool.tile(sh2, f32, tag=tag, name=f"upd_{tag}")

                ccs = w2("u0"); nc.vector.tensor_tensor(ccs[:], c_lt, c_rb, op=op.add)
                cc = w2("u1"); nc.vector.tensor_scalar_mul(cc[:], ccs[:], 0.5)
                gcs = w2("u0"); nc.vector.tensor_tensor(gcs[:], g_lt, g_rb, op=op.add)
                gc = w2("u2"); nc.vector.tensor_scalar_mul(gc[:], gcs[:], 0.5)
                dc = w2("u0"); nc.vector.tensor_tensor(dc[:], gc[:], cc[:], op=op.subtract)
                t045 = w2("u1"); nc.vector.tensor_scalar_mul(t045[:], dc[:], 0.45)
                bsz = w2("u2"); nc.vector.tensor_tensor(bsz[:], c_rb, c_lt, op=op.subtract)
                gsz = w2("u3"); nc.vector.tensor_tensor(gsz[:], g_rb, g_lt, op=op.subtract)
                ds = w2("u2"); nc.vector.tensor_tensor(ds[:], gsz[:], bsz[:], op=op.subtract)
                e = w2("u3"); nc.vector.tensor_tensor(e[:], ds[:], dc[:], op=op.subtract)
                e4 = w2("u0"); nc.vector.tensor_scalar_mul(e4[:], e[:], 0.4)
                nb = bpool.tile([P, BPC * T, 4], f32)
                nc.vector.tensor_tensor(nb[:, :, 0:2], c_lt, t045[:], op=op.add)
                t1 = w2("u2"); nc.vector.tensor_tensor(t1[:], c_rb, t045[:], op=op.add)
                nc.vector.tensor_tensor(nb[:, :, 2:4], t1[:], e4[:], op=op.add)

                nbv = nb[:].rearrange("p (b t) c -> p b t c", b=BPC)
                for b in range(BPC):
                    nc.sync.dma_start(
                        obox[b, it].rearrange("(t p) c -> p t c", p=P), nbv[:, b]
                    )
                cur = nb

    nc.compile()
    return nc


def _prep_core_inputs(boxes, gt):
    """boxes [BPC,N,4], gt [BPC,M,4] -> device input dict (all float32 C-order)."""
    bx = np.ascontiguousarray(
        boxes.reshape(BPC, T, P, 4).transpose(2, 0, 1, 3).reshape(P, BPC * T, 4),
        dtype=np.float32,
    )
    area_b = (gt[:, :, 2] - gt[:, :, 0]) * (gt[:, :, 3] - gt[:, :, 1])  # [BPC, M]
    gb_row = np.concatenate([gt.transpose(0, 2, 1), area_b[:, None, :]], axis=1)
    gbt = np.ascontiguousarray(
        np.broadcast_to(gb_row[None], (P, BPC, 5, M)), dtype=np.float32
    )
    gt5 = np.concatenate(
        [gt.transpose(1, 0, 2), np.ones((M, BPC, 1), np.float32)], axis=2
    )
    gt5 = np.ascontiguousarray(gt5, dtype=np.float32)
    gt0 = np.ascontiguousarray(
        np.broadcast_to(gt[None, :, None, 0, :], (P, BPC, T, 4)).reshape(P, BPC * T, 4),
        dtype=np.float32,
    )
    return {"bx": bx, "gbt": gbt, "gt5": gt5, "gt0": gt0}


def kernel(boxes, gt_boxes):
    from concourse.bass_utils import run_bass_kernel_spmd

    boxes = np.asarray(boxes, dtype=np.float32)
    gt_boxes = np.asarray(gt_boxes, dtype=np.float32)

    if "nc" not in _CACHE:
        _CACHE["nc"] = _build_module()
    nc = _CACHE["nc"]

    in_maps = [
        _prep_core_inputs(
            boxes[c * BPC : (c + 1) * BPC], gt_boxes[c * BPC : (c + 1) * BPC]
        )
        for c in range(NCORES)
    ]
    res = run_bass_kernel_spmd(nc, in_maps, core_ids=list(range(NCORES)))

    all_sims = np.concatenate([r["sims"] for r in res.results], axis=0)
    dev_boxes = np.concatenate([r["obox"] for r in res.results], axis=0)
    all_boxes = np.concatenate([boxes[:, None], dev_boxes], axis=1)
    return all_boxes, all_sims


# revision 16
# speedup vs baseline: 1.2118x; 1.2118x over previous
"""Trainium2 Bass kernel for batched progressive box adjustment (nms_detection).

Contract: kernel(boxes, gt_boxes) -> (all_boxes [B,6,N,4], all_sims [B,6,N,M]),
matching reference.reference(). Self-contained; shards batch B=64 across 8
NeuronCores (8 images per core).

Per-core layout: n on partitions (P=125, T=8 blocks, n = t*125 + p), m on the
free dim. IoU is computed in image-quarters with the x-chain on VectorE and the
y-chain on GpSimd; argmax+gather runs as mask -> PE transpose -> gather matmul
(gt table with a ones column for tie detection) -> PE transpose back. Box
update follows the reference op order exactly; division is the correctly
rounded HW reciprocal + multiply (verified 0 argmax flips vs reference).
Quarters pipeline across iterations via independent per-quarter box tiles.
"""

import numpy as np

B, N, M = 64, 1000, 100
ITERS = 5
NCORES = 8
BPC = B // NCORES      # images per core
P = 125                # SBUF partitions used (N = T * P)
T = 8                  # n-blocks per image
NIT = ITERS + 1        # sims slots
NH = 4                 # image-quarters per core
HB = BPC // NH         # images per quarter

_CACHE = {}


def _build_module(cfg=None):
    import concourse.bacc as bacc
    import concourse.tile as tile
    import concourse.mybir as mybir
    from concourse.alu_op_type import AluOpType as op
    from concourse.masks import make_identity

    cfg = cfg or {}
    USE_GPSIMD = cfg.get("gpsimd", False)
    POOL_SET = cfg.get("pool_set", {"m1y", "m2y", "w0y", "s"})
    if not USE_GPSIMD:
        POOL_SET = set()
    NITL = cfg.get("nit", NIT)

    f32 = mybir.dt.float32
    X = mybir.AxisListType.X
    Relu = mybir.ActivationFunctionType.Relu

    nc = bacc.Bacc()
    bx_in = nc.dram_tensor("bx", [P, BPC * T, 4], f32, kind="ExternalInput")
    gbt_in = nc.dram_tensor("gbt", [P, BPC, 5, M], f32, kind="ExternalInput")
    gt5_in = nc.dram_tensor("gt5", [M, BPC, 5], f32, kind="ExternalInput")
    gt0_in = nc.dram_tensor("gt0", [P, BPC * T, 4], f32, kind="ExternalInput")
    sims = nc.dram_tensor("sims", [BPC, NIT, N, M], f32, kind="ExternalOutput")
    obox = nc.dram_tensor("obox", [BPC, ITERS, N, 4], f32, kind="ExternalOutput")

    def eng(name):
        return nc.gpsimd if name in POOL_SET else nc.vector

    with tile.TileContext(nc) as tc:
        with (
            tc.tile_pool(name="const", bufs=1) as cpool,
            tc.tile_pool(name="boxes", bufs=2) as bpool,
            tc.tile_pool(name="work", bufs=1) as wpool,
            tc.tile_pool(name="iou", bufs=2) as ipool,
            tc.tile_pool(name="maskp", bufs=2) as mpool,
            tc.tile_pool(name="stage", bufs=2) as spool,
            tc.tile_pool(name="small", bufs=2) as tpool,
            tc.tile_pool(name="ps_mT", bufs=2, space="PSUM") as ps_mT,
            tc.tile_pool(name="ps_bgT", bufs=1, space="PSUM") as ps_bgT,
            tc.tile_pool(name="ps_bg", bufs=2, space="PSUM") as ps_bg,
        ):
            ident = cpool.tile([128, 128], f32)
            make_identity(nc, ident[:])
            G = cpool.tile([P, BPC, 5, M], f32)
            nc.sync.dma_start(G[:], gbt_in[:])
            GT5 = cpool.tile([M, BPC, 5], f32)
            nc.sync.dma_start(GT5[:], gt5_in[:])
            GT0 = cpool.tile([P, BPC * T, 4], f32)
            nc.sync.dma_start(GT0[:], gt0_in[:])

            # per-quarter current box tiles (independent cross-iter pipelines)
            cur = []
            for h in range(NH):
                cb_t = bpool.tile([P, HB * T, 4], f32, tag=f"cur{h}", name=f"cur{h}")
                nc.sync.dma_start(cb_t[:], bx_in[:, h * HB * T : (h + 1) * HB * T, :])
                cur.append(cb_t)

            for it in range(NITL):
                for h in range(NH):
                    bs = slice(h * HB, (h + 1) * HB)
                    sh = [P, HB, T, M]
                    ch = cur[h]

                    # per-box areas for this quarter, columnar [P, HB*T]
                    wa = tpool.tile([P, HB * T], f32, tag="wa", name="wa")
                    nc.vector.tensor_tensor(wa[:], ch[:, :, 2], ch[:, :, 0], op=op.subtract)
                    ha = tpool.tile([P, HB * T], f32, tag="ha", name="ha")
                    nc.vector.tensor_tensor(ha[:], ch[:, :, 3], ch[:, :, 1], op=op.subtract)
                    areaA = tpool.tile([P, HB * T], f32, tag="areaA", name="areaA")
                    nc.vector.tensor_tensor(areaA[:], wa[:], ha[:], op=op.mult)

                    def gb(j):
                        return G[:, bs, j, :].unsqueeze(2).broadcast_to(sh)

                    def cbb(c):
                        v = ch[:, :, c].rearrange("p (b t) -> p b t", b=HB)
                        return v.unsqueeze(3).broadcast_to(sh)

                    aab = areaA[:].rearrange("p (b t) -> p b t", b=HB)
                    aab = aab.unsqueeze(3).broadcast_to(sh)

                    m1x = wpool.tile(sh, f32, tag="w0", name="m1x")
                    nc.vector.tensor_tensor(m1x[:], gb(2), cbb(2), op=op.min)
                    m2x = wpool.tile(sh, f32, tag="w1", name="m2x")
                    nc.vector.tensor_tensor(m2x[:], gb(0), cbb(0), op=op.max)
                    w0x = wpool.tile(sh, f32, tag="w2", name="w0x")
                    nc.vector.tensor_tensor(w0x[:], m1x[:], m2x[:], op=op.subtract)
                    wrx = wpool.tile(sh, f32, tag="w3", name="wrx")
                    nc.scalar.activation(wrx[:], w0x[:], Relu)
                    m1y = wpool.tile(sh, f32, tag="w4", name="m1y")
                    eng("m1y").tensor_tensor(m1y[:], gb(3), cbb(3), op=op.min)
                    m2y = wpool.tile(sh, f32, tag="w5", name="m2y")
                    eng("m2y").tensor_tensor(m2y[:], gb(1), cbb(1), op=op.max)
                    w0y = wpool.tile(sh, f32, tag="w6", name="w0y")
                    eng("w0y").tensor_tensor(w0y[:], m1y[:], m2y[:], op=op.subtract)
                    s = wpool.tile(sh, f32, tag="w7", name="s")
                    eng("s").tensor_tensor(s[:], aab, gb(4), op=op.add)
                    inter = wpool.tile(sh, f32, tag="w0", name="inter")
                    nc.vector.scalar_tensor_tensor(
                        inter[:], w0y[:], 0.0, wrx[:], op0=op.max, op1=op.mult
                    )
                    union = wpool.tile(sh, f32, tag="w1", name="union")
                    nc.vector.scalar_tensor_tensor(
                        union[:], inter[:], -1.0, s[:], op0=op.mult, op1=op.add
                    )
                    recip = wpool.tile(sh, f32, tag="w2", name="recip")
                    nc.vector.reciprocal(recip[:], union[:])
                    iou = ipool.tile(sh, f32, tag="iou", name="iou")
                    eng("q").tensor_tensor(iou[:], inter[:], recip[:], op=op.mult)

                    for bq in range(HB):
                        nc.sync.dma_start(
                            sims[h * HB + bq, it].rearrange("(t p) m -> p t m", p=P),
                            iou[:, bq],
                        )

                    if it == NITL - 1:
                        continue

                    # --- argmax + gather via PE for this quarter ---
                    rmax = tpool.tile([P, HB, T], f32, tag="rmax", name="rmax")
                    nc.vector.reduce_max(rmax[:], iou[:], axis=X)
                    mask = mpool.tile(sh, f32, tag="mask", name="mask")
                    eng("mask").tensor_tensor(
                        mask[:],
                        iou[:],
                        rmax[:].unsqueeze(3).broadcast_to(sh),
                        op=op.is_ge,
                    )

                    bg_ps = ps_bg.tile([P, HB, T, 5], f32, tag="bg", name="bg_ps")
                    for bq in range(HB):
                        b = h * HB + bq
                        mT_ps = ps_mT.tile([M, T, 128], f32, tag="mT", name="mT_ps")
                        for t in range(T):
                            nc.tensor.transpose(
                                mT_ps[:, t, :P], mask[:, bq, t, :], ident[:P, :P]
                            )
                        mT = spool.tile([M, T, 128], f32, tag="mT", name="mT")
                        nc.scalar.copy(mT[:], mT_ps[:])
                        bgT_ps = ps_bgT.tile([5, T, 128], f32, tag="bgT", name="bgT_ps")
                        for t in range(T):
                            nc.tensor.matmul(
                                bgT_ps[:, t, :P], GT5[:, b, :], mT[:, t, :P],
                                start=True, stop=True,
                            )
                        bgT = spool.tile([5, T, 128], f32, tag="bgT", name="bgT")
                        nc.scalar.copy(bgT[:], bgT_ps[:])
                        for t in range(T):
                            nc.tensor.transpose(
                                bg_ps[:, bq, t, :], bgT[:, t, :P], ident[:5, :5]
                            )
                    bg = tpool.tile([P, HB, T, 5], f32, tag="bg", name="bg")
                    nc.scalar.copy(bg[:], bg_ps[:])

                    # --- tie fix: all-zero rows take gt[0] ---
                    sel = tpool.tile([P, HB, T], mybir.dt.int32, tag="sel", name="sel")
                    nc.vector.tensor_scalar(
                        sel[:], bg[:, :, :, 4], 1.5, None, op0=op.is_gt
                    )
                    g4 = tpool.tile([P, HB * T, 4], f32, tag="g4", name="g4")
                    g4v = g4[:].rearrange("p (b t) c -> p b t c", b=HB)
                    nc.vector.tensor_copy(g4v, bg[:, :, :, 0:4])
                    nc.vector.copy_predicated(
                        g4v,
                        sel[:].unsqueeze(3).broadcast_to([P, HB, T, 4]),
                        GT0[:, h * HB * T : (h + 1) * HB * T, :].rearrange(
                            "p (b t) c -> p b t c", b=HB
                        ),
                    )

                    # --- box update (reference op order), (x,y) pairs batched ---
                    c_lt = ch[:, :, 0:2]
                    c_rb = ch[:, :, 2:4]
                    g_lt = g4[:, :, 0:2]
                    g_rb = g4[:, :, 2:4]
                    sh2 = [P, HB * T, 2]

                    def w2(tag):
                        return tpool.tile(sh2, f32, tag=tag, name=f"upd_{tag}")

                    ccs = w2("u0"); nc.vector.tensor_tensor(ccs[:], c_lt, c_rb, op=op.add)
                    cc = w2("u1"); nc.vector.tensor_scalar_mul(cc[:], ccs[:], 0.5)
                    gcs = w2("u0"); nc.vector.tensor_tensor(gcs[:], g_lt, g_rb, op=op.add)
                    gc = w2("u2"); nc.vector.tensor_scalar_mul(gc[:], gcs[:], 0.5)
                    dc = w2("u0"); nc.vector.tensor_tensor(dc[:], gc[:], cc[:], op=op.subtract)
                    t045 = w2("u1"); nc.vector.tensor_scalar_mul(t045[:], dc[:], 0.45)
                    bsz = w2("u2"); nc.vector.tensor_tensor(bsz[:], c_rb, c_lt, op=op.subtract)
                    gsz = w2("u3"); nc.vector.tensor_tensor(gsz[:], g_rb, g_lt, op=op.subtract)
                    ds = w2("u2"); nc.vector.tensor_tensor(ds[:], gsz[:], bsz[:], op=op.subtract)
                    e = w2("u3"); nc.vector.tensor_tensor(e[:], ds[:], dc[:], op=op.subtract)
                    e4 = w2("u0"); nc.vector.tensor_scalar_mul(e4[:], e[:], 0.4)
                    nb = bpool.tile([P, HB * T, 4], f32, tag=f"cur{h}", name=f"nb{h}")
                    nc.vector.tensor_tensor(nb[:, :, 0:2], c_lt, t045[:], op=op.add)
                    t1 = w2("u2"); nc.vector.tensor_tensor(t1[:], c_rb, t045[:], op=op.add)
                    nc.vector.tensor_tensor(nb[:, :, 2:4], t1[:], e4[:], op=op.add)

                    nbv = nb[:].rearrange("p (b t) c -> p b t c", b=HB)
                    for bq in range(HB):
                        nc.sync.dma_start(
                            obox[h * HB + bq, it].rearrange("(t p) c -> p t c", p=P),
                            nbv[:, bq],
                        )
                    cur[h] = nb

    nc.compile()
    return nc


def _prep_core_inputs(boxes, gt):
    """boxes [BPC,N,4], gt [BPC,M,4] -> device input dict (all float32 C-order)."""
    bx = np.ascontiguousarray(
        boxes.reshape(BPC, T, P, 4).transpose(2, 0, 1, 3).reshape(P, BPC * T, 4),
        dtype=np.float32,
    )
    area_b = (gt[:, :, 2] - gt[:, :, 0]) * (gt[:, :, 3] - gt[:, :, 1])  # [BPC, M]
    gb_row = np.concatenate([gt.transpose(0, 2, 1), area_b[:, None, :]], axis=1)
    gbt = np.ascontiguousarray(
        np.broadcast_to(gb_row[None], (P, BPC, 5, M)), dtype=np.float32
    )
    gt5 = np.concatenate(
        [gt.transpose(1, 0, 2), np.ones((M, BPC, 1), np.float32)], axis=2
    )
    gt5 = np.ascontiguousarray(gt5, dtype=np.float32)
    gt0 = np.ascontiguousarray(
        np.broadcast_to(gt[None, :, None, 0, :], (P, BPC, T, 4)).reshape(P, BPC * T, 4),
        dtype=np.float32,
    )
    return {"bx": bx, "gbt": gbt, "gt5": gt5, "gt0": gt0}


def kernel(boxes, gt_boxes):
    from concourse.bass_utils import run_bass_kernel_spmd

    boxes = np.asarray(boxes, dtype=np.float32)
    gt_boxes = np.asarray(gt_boxes, dtype=np.float32)

    if "nc" not in _CACHE:
        _CACHE["nc"] = _build_module()
    nc = _CACHE["nc"]

    in_maps = [
        _prep_core_inputs(
            boxes[c * BPC : (c + 1) * BPC], gt_boxes[c * BPC : (c + 1) * BPC]
        )
        for c in range(NCORES)
    ]
    res = run_bass_kernel_spmd(nc, in_maps, core_ids=list(range(NCORES)))

    all_sims = np.concatenate([r["sims"] for r in res.results], axis=0)
    dev_boxes = np.concatenate([r["obox"] for r in res.results], axis=0)
    all_boxes = np.concatenate([boxes[:, None], dev_boxes], axis=1)
    return all_boxes, all_sims
